# revision 12
# baseline (speedup 1.0000x reference)
"""Self-contained Trainium2 Bass kernel for nn_ChamferVAELoss.

kernel(**inputs) takes the FULL inputs (output[16,1024,8], mu[16,128],
log_var[16,128], real[16,1024,8]) and returns the full reference output
(shape [1] float32). The batch dimension (16) is sharded 2-per-core across
8 NeuronCores; each core computes, per batch, the negated scaled pairwise
smooth-L1 matrix se8n = W - ||x-y||^2 via TensorE matmuls + ACT/DVE
elementwise work, reduces row/col maxima on-device, and returns [128]
partial sums per batch. The host sums partials and scales:
  chamfer = -(sum of partials) / (16 * B * N).
The 0.0 * mean(dkl) term in the reference contributes exactly 0 for finite
inputs, so mu/log_var only participate via finiteness (assumed).
"""
"""Chamfer smooth-L1 loss kernel (per-core program builder).

Math (per batch):
  smoothl1(x) = 0.5 x^2 - 0.5 relu(|x|-1)^2
  se8n[np, nt] := -16 * mean_d smoothl1(pred[np,d]-real[nt,d])
               = W - sqL2,  W = sum_d relu(|t_d|-1)^2,  sqL2 = sum_d t_d^2
  rowmax/colmax of se8n = -16 * (rowmin/colmin of se).
  Output: res[128, SB]; host sums partitions: total_b = -16 * loss_sum_b.

All matmuls use bf16 operands (fp32 matmul is ~30x slower on this HW) with
hi/lo pair decomposition for near-fp32 accuracy:
  diff:  K=4  [p_hi; p_lo; 1; 1] x [1; 1; -r_hi; -r_lo]  (exact diff)
  sqL2:  K=28 [np_hi; np_lo; 1; 1; m2p_hi x8; m2p_hi x8; m2p_lo x8]
            x [1; 1; nr_hi; nr_lo; r_hi x8; r_lo x8; r_hi x8]
         (= np - 2 p.r + nr, dropping only the lo*lo cross term)
  norms: K=8 ones8 x sq_hi + ones8 x sq_lo (PSUM-accumulated)

Engines: PE diffs/sqL2/transposes; ACT |t| evac + (-sqL2) evac + a share of
squares; DVE relu shift (dual ts, bf16 4x), squares (bf16 2x), d-sum,
row/col max.
"""
import numpy as np
from contextlib import ExitStack
import concourse.bass as bass
import concourse.mybir as mybir

F32 = mybir.dt.float32
BF16 = mybir.dt.bfloat16
AX = mybir.AxisListType
OP = mybir.AluOpType
AF = mybir.ActivationFunctionType

SB = 2
N = 1024
D = 8
G = 8
NCHUNK = 512


class _FakeInst:
    def then_inc(self, *a, **k):
        return self


class _FakeEngine:
    def __getattr__(self, name):
        return lambda *a, **k: _FakeInst()


class Sched:
    def __init__(self, sems, dry, ev=None):
        self.sems = sems
        self.dry = dry
        self.count = {k: 0 for k in ["dma", "pe", "act", "dve", "pool"]}
        self.ev = {} if ev is None else ev
        self.pfx = ""

    def emit(self, engine_name, inst, inc, event=None):
        if not self.dry:
            inst.then_inc(self.sems[engine_name], inc)
        self.count[engine_name] += inc
        if event is not None:
            event = self.pfx + event
            if self.dry:
                self.ev[event] = (engine_name, self.count[engine_name])
            else:
                assert self.ev[event] == (engine_name, self.count[engine_name]), event
        return inst

    def wait(self, engine, event, pfx=None):
        if self.dry:
            return
        src, val = self.ev[(self.pfx if pfx is None else pfx) + event]
        engine.wait_ge(self.sems[src], val)

    def alias(self, new, old):
        if self.dry:
            self.ev[self.pfx + new] = self.ev[self.pfx + old]

    def alias_abs(self, new, abs_event):
        """Alias current-prefix event `new` to an absolute (already-prefixed)
        earlier event, e.g. a rep-0 constant-load completion."""
        if self.dry:
            self.ev[self.pfx + new] = self.ev[abs_event]

    def mark(self, engine_name, event):
        """Record the current count of `engine_name` under an event name
        without emitting an instruction (end-of-rep barrier bookkeeping)."""
        if self.dry:
            self.ev[self.pfx + event] = (engine_name, self.count[engine_name])


def uidx(b, g, d):
    return (b * G + g) * D + d


def make_act_sq(frac):
    sel = [False] * (SB * G * D)
    k = 0.0
    for b in range(SB):
        for g in range(G):
            for d in range(1, D):
                k += frac * 8.0 / 7.0
                if k >= 1.0:
                    sel[uidx(b, g, d)] = True
                    k -= 1.0
    return sel


def build_chamfer_nc(act_square_frac=0.5, debug=False, reps=1):
    nc = bass.Bass()
    pred = nc.dram_tensor("pred", [SB, D, N], F32, kind="ExternalInput")
    real = nc.dram_tensor("real", [SB, D, N], F32, kind="ExternalInput")
    idb = nc.dram_tensor("idb", [128, 128], BF16, kind="ExternalInput")
    cstb = nc.dram_tensor("cstb", [1, N], BF16, kind="ExternalInput")
    res = [nc.dram_tensor(f"res{b}", [128, 1], F32, kind="ExternalOutput")
           for b in range(SB)]
    if debug:
        dbg_rmax = nc.dram_tensor("dbg_rmax", [128, 16], F32, kind="ExternalOutput")
        dbg_cmax = nc.dram_tensor("dbg_cmax", [128, 16], F32, kind="ExternalOutput")
        dbg_cm0 = nc.dram_tensor("dbg_cm0", [128, N], F32, kind="ExternalOutput")
        dbg_cm1 = nc.dram_tensor("dbg_cm1", [128, N], F32, kind="ExternalOutput")
        dbg_nsq = nc.dram_tensor("dbg_nsq", [128, N], F32, kind="ExternalOutput")
        dbg_acc = nc.dram_tensor("dbg_acc", [128, N], F32, kind="ExternalOutput")

    es = ExitStack()
    sbuf = lambda name, shape, dt=F32: es.enter_context(nc.sbuf_tensor(name, shape, dt))

    predTaug = [sbuf(f"predTaug{d}", [4, N], BF16) for d in range(D)]
    realTaug = [sbuf(f"realTaug{d}", [4, N], BF16) for d in range(D)]
    sqlhsT = sbuf("sqlhsT", [28, N], BF16)
    sqrhsT = sbuf("sqrhsT", [28, N], BF16)
    predT8 = sbuf("predT8", [8, N])
    realT8 = sbuf("realT8", [8, N])
    sqscr = sbuf("sqscr", [8, N])
    p_hi = sbuf("p_hi", [8, N], BF16)
    p_lo = sbuf("p_lo", [8, N], BF16)
    r_hi = sbuf("r_hi", [8, N], BF16)
    r_lo = sbuf("r_lo", [8, N], BF16)
    mr_hi = sbuf("mr_hi", [8, N], BF16)
    mr_lo = sbuf("mr_lo", [8, N], BF16)
    m2p_hi = sbuf("m2p_hi", [8, N], BF16)
    m2p_lo = sbuf("m2p_lo", [8, N], BF16)
    sq_hi = sbuf("sq_hi", [8, N], BF16)
    sq_lo = sbuf("sq_lo", [8, N], BF16)
    n_hi = sbuf("n_hi", [1, N], BF16)
    n_lo = sbuf("n_lo", [1, N], BF16)
    ones8b = sbuf("ones8b", [8, 1], BF16)
    sidb = sbuf("sidb", [128, 128], BF16)
    a_t = [sbuf(f"a{i}", [128, N], BF16) for i in range(3)]
    r_t = [sbuf(f"r{i}", [128, N], BF16) for i in range(2)]
    w_t = [sbuf(f"w{i}", [128, N], BF16) for i in range(2)]
    acc_t = [sbuf(f"acc{i}", [128, N], BF16) for i in range(2)]
    nsq_t = [sbuf(f"nsq{i}", [128, N], BF16) for i in range(2)]
    se_t = [sbuf(f"se{i}", [128, N], BF16) for i in range(2)]
    cmax = [sbuf(f"cmaxb{b}", [128, N], BF16) for b in range(SB)]
    if debug:
        dbg_cm_f = [sbuf(f"dbgcm{b}", [128, N]) for b in range(SB)]
        dbg_nsq_f = sbuf("dbgnsq", [128, N])
        dbg_acc_f = sbuf("dbgacc", [128, N])
    rmaxc = sbuf("rmaxc", [128, 16])
    cmaxc = sbuf("cmaxc", [128, 16])
    ra = [sbuf(f"ra{b}", [128, 1]) for b in range(SB)]
    rc = [sbuf(f"rc{b}", [128, 1]) for b in range(SB)]
    res_b = [sbuf(f"resb{b}", [128, 1]) for b in range(SB)]

    pt = [es.enter_context(nc.psum_tensor(f"pt{i}", [128, N], F32)) for i in range(3)]
    pq = es.enter_context(nc.psum_tensor("pq", [128, N], F32))

    sems = {
        "dma": es.enter_context(nc.semaphore("dma_sem")),
        "pe": es.enter_context(nc.semaphore("t_sem")),
        "act": es.enter_context(nc.semaphore("a_sem")),
        "dve": es.enter_context(nc.semaphore("v_sem")),
    }

    act_sq = make_act_sq(act_square_frac)
    NU = SB * G * D

    ENGS = ["dma", "pe", "act", "dve"]

    def barrier(S, engine, rep, self_name):
        if rep > 0:
            for e in ENGS:
                if e != self_name:
                    S.wait(engine, f"end_{e}", pfx=f"R{rep - 1}_")

    # ---------------- engine bodies ----------------
    def body_sync(sync, S, rep=0):
        barrier(S, sync, rep, "dma")
        if rep > 0:
            S.alias_abs("dma_idb", "R0_dma_idb")
            S.alias_abs("dma_ones", "R0_dma_ones")
        else:
            S.emit("dma", sync.dma_start(sidb[:], idb[:]), 16, "dma_idb")
            for d in range(D):
                for rr in (2, 3):
                    S.emit("dma", sync.dma_start(predTaug[d][rr:rr + 1, :], cstb[:]), 16)
                for rr in (0, 1):
                    S.emit("dma", sync.dma_start(realTaug[d][rr:rr + 1, :], cstb[:]), 16)
            for rr in (2, 3):
                S.emit("dma", sync.dma_start(sqlhsT[rr:rr + 1, :], cstb[:]), 16)
            for rr in (0, 1):
                ev = "dma_ones" if rr == 1 else None
                S.emit("dma", sync.dma_start(sqrhsT[rr:rr + 1, :], cstb[:]), 16, ev)
        for b in range(SB):
            if b > 0:
                S.wait(sync, f"mm_diff{uidx(b - 1, G - 1, D - 1)}")
                S.wait(sync, f"nr_split{b-1}")
            S.emit("dma", sync.dma_start(predT8[:], pred[b]), 16, f"dma_predT8{b}")
            S.emit("dma", sync.dma_start(realT8[:], real[b]), 16, f"dma_realT8{b}")
            S.wait(sync, f"p_split{b}")
            for d in range(D):
                S.emit("dma", sync.dma_start(predTaug[d][0:1, :], p_hi[d:d + 1, :]), 16)
                S.emit("dma", sync.dma_start(predTaug[d][1:2, :], p_lo[d:d + 1, :]), 16)
            S.wait(sync, f"m2p_done{b}")
            S.emit("dma", sync.dma_start(sqlhsT[4:12, :], m2p_hi[:]), 16)
            S.emit("dma", sync.dma_start(sqlhsT[12:20, :], m2p_hi[:]), 16)
            S.emit("dma", sync.dma_start(sqlhsT[20:28, :], m2p_lo[:]), 16,
                   f"dma_sqlhsT_m{b}")
            S.wait(sync, f"mr_done{b}")
            for d in range(D):
                S.emit("dma", sync.dma_start(realTaug[d][2:3, :], mr_hi[d:d + 1, :]), 16)
                ev = f"dma_realTaug{b}" if d == D - 1 else None
                S.emit("dma", sync.dma_start(realTaug[d][3:4, :], mr_lo[d:d + 1, :]), 16, ev)
            S.wait(sync, f"r_split{b}")
            S.emit("dma", sync.dma_start(sqrhsT[4:12, :], r_hi[:]), 16)
            S.emit("dma", sync.dma_start(sqrhsT[12:20, :], r_lo[:]), 16)
            S.emit("dma", sync.dma_start(sqrhsT[20:28, :], r_hi[:]), 16,
                   f"dma_sqrhsT_r{b}")
            S.wait(sync, f"np_split{b}")
            S.emit("dma", sync.dma_start(sqlhsT[0:1, :], n_hi[:]), 16)
            S.emit("dma", sync.dma_start(sqlhsT[1:2, :], n_lo[:]), 16,
                   f"dma_np{b}")
            S.wait(sync, f"nr_split{b}")
            S.emit("dma", sync.dma_start(sqrhsT[2:3, :], n_hi[:]), 16)
            S.emit("dma", sync.dma_start(sqrhsT[3:4, :], n_lo[:]), 16,
                   f"dma_nr{b}")
        S.wait(sync, "res_ready")
        for b in range(SB):
            S.emit("dma", sync.dma_start(res[b][:], res_b[b][:]), 16,
                   "dma_out" if b == SB - 1 else None)
        S.mark("dma", "end_dma")
        if debug:
            S.emit("dma", sync.dma_start(dbg_rmax[:], rmaxc[:]), 16)
            S.emit("dma", sync.dma_start(dbg_cmax[:], cmaxc[:]), 16)
            S.emit("dma", sync.dma_start(dbg_cm0[:], dbg_cm_f[0][:]), 16)
            S.emit("dma", sync.dma_start(dbg_cm1[:], dbg_cm_f[1][:]), 16)
            S.emit("dma", sync.dma_start(dbg_nsq[:], dbg_nsq_f[:]), 16)
            S.emit("dma", sync.dma_start(dbg_acc[:], dbg_acc_f[:]), 16)

    def body_tensor(tensor, S, rep=0):
        barrier(S, tensor, rep, "pe")
        for b in range(SB):
            S.wait(tensor, f"sqp_split{b}")
            S.wait(tensor, "ones_done")
            if b > 0:
                S.wait(tensor, f"act_nsq{b-1}_{G-1}")  # pq free
            last = None
            for c in range(2):
                cs = slice(c * NCHUNK, (c + 1) * NCHUNK)
                tensor.matmul(pq[0:1, cs], ones8b[:], sq_hi[:, cs],
                              start=True, stop=False)
                last = tensor.matmul(pq[0:1, cs], ones8b[:], sq_lo[:, cs],
                                     start=False, stop=True)
            S.emit("pe", last, 1, f"mm_np{b}")
            S.wait(tensor, f"sqr_split{b}")
            S.wait(tensor, f"np_split{b}")
            for c in range(2):
                cs = slice(c * NCHUNK, (c + 1) * NCHUNK)
                tensor.matmul(pq[0:1, cs], ones8b[:], sq_hi[:, cs],
                              start=True, stop=False)
                last = tensor.matmul(pq[0:1, cs], ones8b[:], sq_lo[:, cs],
                                     start=False, stop=True)
            S.emit("pe", last, 1, f"mm_nr{b}")

            for g in range(G):
                gs = slice(g * 128, (g + 1) * 128)
                if g == 0:
                    S.wait(tensor, f"dma_nr{b}")
                else:
                    S.wait(tensor, f"act_nsq{b}_{g-1}")
                last = None
                for c in range(2):
                    cs = slice(c * NCHUNK, (c + 1) * NCHUNK)
                    last = tensor.matmul(pq[:, cs], sqlhsT[:, gs], sqrhsT[:, cs],
                                         start=True, stop=True)
                S.emit("pe", last, 1, f"mm_sq{b}_{g}")

                for d in range(D):
                    u = uidx(b, g, d)
                    s = u % 3
                    if u >= 3:
                        S.wait(tensor, f"act_abs{u-3}")
                    if d == 0 and g == 0:
                        S.wait(tensor, f"dma_realTaug{b}")
                    last = None
                    for c in range(2):
                        cs = slice(c * NCHUNK, (c + 1) * NCHUNK)
                        last = tensor.matmul(pt[s][:, cs],
                                             predTaug[d][:, gs],
                                             realTaug[d][:, cs],
                                             start=True, stop=True)
                    S.emit("pe", last, 1, f"mm_diff{u}")

        S.wait(tensor, "dma_idb")
        S.wait(tensor, f"act_abs{NU-1}")
        for b in range(SB):
            S.wait(tensor, f"cmax_done{b}")
            for j in range(8):
                jj = b * 8 + j
                if jj >= 3:
                    S.wait(tensor, f"cred{jj-3}")
                ptb = pt[jj % 3].ap().bitcast(BF16)
                last = tensor.transpose(ptb[:, 0:128],
                                        cmax[b][:, j * 128:(j + 1) * 128],
                                        sidb[:])
                S.emit("pe", last, 1, f"mm_tr{jj}")
        S.mark("pe", "end_pe")

    def body_scalar(scalar, S, rep=0):
        barrier(S, scalar, rep, "act")
        for b in range(SB):
            for g in range(G):
                gi = b * G + g
                S.wait(scalar, f"mm_sq{b}_{g}")
                if gi >= 2:
                    S.wait(scalar, f"se_done{gi-2}")
                inst = scalar.activation(nsq_t[gi % 2][:], pq[:], AF.Copy,
                                         bias=0.0, scale=-1.0)
                S.emit("act", inst, 1, f"act_nsq{b}_{g}")
                for d in range(D):
                    u = uidx(b, g, d)
                    S.wait(scalar, f"mm_diff{u}")
                    if u >= 3:
                        S.wait(scalar, f"dve_r{u-3}")
                    inst = scalar.activation(a_t[u % 3][:], pt[u % 3][:], AF.Abs)
                    S.emit("act", inst, 1, f"act_abs{u}")
                    if act_sq[u]:
                        S.wait(scalar, f"dve_r{u}")
                        inst = scalar.activation(w_t[u % 2][:], r_t[u % 2][:],
                                                 AF.Square)
                        S.emit("act", inst, 1, f"sq{u}")
        S.mark("act", "end_act")

    def body_vector(vector, S, rep=0):
        barrier(S, vector, rep, "dve")
        S.emit("dve", vector.memset(ones8b[:], 1.0), 1, "ones_done")

        for b in range(SB):
            S.wait(vector, f"dma_predT8{b}")
            if b > 0:
                S.wait(vector, f"dma_sqlhsT_m{b-1}")
            S.emit("dve", vector.tensor_copy(p_hi[:], predT8[:]), 1)
            S.emit("dve", vector.tensor_tensor(p_lo[:], predT8[:], p_hi[:],
                                               OP.subtract), 1, f"p_split{b}")
            S.emit("dve", vector.tensor_scalar(m2p_hi[:], p_hi[:], -2.0, None,
                                               OP.mult), 1)
            S.emit("dve", vector.tensor_scalar(m2p_lo[:], p_lo[:], -2.0, None,
                                               OP.mult), 1, f"m2p_done{b}")
            S.wait(vector, f"dma_realT8{b}")
            if b > 0:
                S.wait(vector, f"dma_sqrhsT_r{b-1}")
            S.emit("dve", vector.tensor_copy(r_hi[:], realT8[:]), 1)
            S.emit("dve", vector.tensor_tensor(r_lo[:], realT8[:], r_hi[:],
                                               OP.subtract), 1, f"r_split{b}")
            S.emit("dve", vector.tensor_scalar(mr_hi[:], r_hi[:], -1.0, None,
                                               OP.mult), 1)
            S.emit("dve", vector.tensor_scalar(mr_lo[:], r_lo[:], -1.0, None,
                                               OP.mult), 1, f"mr_done{b}")
            S.emit("dve", vector.tensor_tensor(sqscr[:], predT8[:], predT8[:],
                                               OP.mult), 1)
            S.emit("dve", vector.tensor_copy(sq_hi[:], sqscr[:]), 1)
            S.emit("dve", vector.tensor_tensor(sq_lo[:], sqscr[:], sq_hi[:],
                                               OP.subtract), 1, f"sqp_split{b}")
            S.wait(vector, f"mm_np{b}")
            S.emit("dve", vector.tensor_copy(n_hi[:], pq[0:1, :]), 1)
            S.emit("dve", vector.tensor_tensor(n_lo[:], pq[0:1, :], n_hi[:],
                                               OP.subtract), 1, f"np_split{b}")
            S.emit("dve", vector.tensor_tensor(sqscr[:], realT8[:], realT8[:],
                                               OP.mult), 1)
            S.emit("dve", vector.tensor_copy(sq_hi[:], sqscr[:]), 1)
            S.emit("dve", vector.tensor_tensor(sq_lo[:], sqscr[:], sq_hi[:],
                                               OP.subtract), 1, f"sqr_split{b}")
            S.wait(vector, f"mm_nr{b}")
            S.wait(vector, f"dma_np{b}")
            S.emit("dve", vector.tensor_copy(n_hi[:], pq[0:1, :]), 1)
            S.emit("dve", vector.tensor_tensor(n_lo[:], pq[0:1, :], n_hi[:],
                                               OP.subtract), 1, f"nr_split{b}")

            for g in range(G):
                gi = b * G + g
                for d in range(D):
                    u = uidx(b, g, d)
                    S.wait(vector, f"act_abs{u}")
                    inst = vector.tensor_scalar(r_t[u % 2][:], a_t[u % 3][:],
                                                1.0, 1.0, OP.max, OP.subtract)
                    S.emit("dve", inst, 1, f"dve_r{u}")
                    if act_sq[u]:
                        S.wait(vector, f"sq{u}")
                        S.emit("dve",
                               vector.tensor_tensor(acc_t[gi % 2][:],
                                                    acc_t[gi % 2][:],
                                                    w_t[u % 2][:], OP.add),
                               1, f"acc{u}")
                    else:
                        tgt = acc_t[gi % 2] if d == 0 else w_t[u % 2]
                        S.emit("dve", vector.tensor_tensor(tgt[:], r_t[u % 2][:],
                                                           r_t[u % 2][:],
                                                           OP.mult), 1, f"sqv{u}")
                        if d > 0:
                            S.emit("dve",
                                   vector.tensor_tensor(acc_t[gi % 2][:],
                                                        acc_t[gi % 2][:],
                                                        w_t[u % 2][:], OP.add),
                                   1, f"acc{u}")
                S.wait(vector, f"act_nsq{b}_{g}")
                if debug and b == 0 and g == 0:
                    S.emit("dve", vector.tensor_copy(dbg_nsq_f[:],
                                                     nsq_t[gi % 2][:]), 1,
                           "dbg_nsq_done")
                    S.emit("dve", vector.tensor_copy(dbg_acc_f[:],
                                                     acc_t[gi % 2][:]), 1,
                           "dbg_acc_done")
                tgt_se = cmax[b] if g == 0 else se_t[gi % 2]
                S.emit("dve", vector.tensor_tensor(tgt_se[:], acc_t[gi % 2][:],
                                                   nsq_t[gi % 2][:], OP.add), 1,
                       f"se_done{gi}")
                S.emit("dve", vector.tensor_reduce(rmaxc[:, gi:gi + 1], tgt_se[:],
                                                   AX.X, OP.max), 1, f"rmax{gi}")
                if g > 0:
                    S.emit("dve", vector.tensor_tensor(cmax[b][:], cmax[b][:],
                                                       tgt_se[:], OP.max), 1,
                           f"cmaxup{gi}")
            S.alias(f"cmax_done{b}", f"cmaxup{b * G + G - 1}")

        for b in range(SB):
            for j in range(8):
                jj = b * 8 + j
                S.wait(vector, f"mm_tr{jj}")
                ptb = pt[jj % 3].ap().bitcast(BF16)
                S.emit("dve", vector.tensor_reduce(cmaxc[:, jj:jj + 1],
                                                   ptb[:, 0:128], AX.X, OP.max),
                       1, f"cred{jj}")
        if debug:
            for b in range(SB):
                S.emit("dve", vector.tensor_copy(dbg_cm_f[b][:], cmax[b][:]), 1,
                       f"dbgcm_done{b}")
        # NB: a DVE op must not read the output of the immediately
        # preceding DVE instruction (reduce writeback races the next op's
        # reads on this toolchain) -- interleave so RAW distance >= 2.
        for b in range(SB):
            S.emit("dve", vector.tensor_reduce(ra[b][:], rmaxc[:, b * 8:b * 8 + 8],
                                               AX.X, OP.add), 1, f"rsum{b}")
            S.emit("dve", vector.tensor_reduce(rc[b][:], cmaxc[:, b * 8:b * 8 + 8],
                                               AX.X, OP.add), 1, f"csum{b}")
        for b in range(SB):
            S.emit("dve", vector.tensor_tensor(res_b[b][:], ra[b][:], rc[b][:],
                                               OP.add), 1, f"tot_ready{b}")
        S.alias("res_ready", f"tot_ready{SB-1}")
        S.mark("dve", "end_dve")

    # ---------------- two passes ----------------
    S0 = Sched(sems, dry=True)
    fake = _FakeEngine()
    for r in range(reps):
        S0.pfx = f"R{r}_"
        body_sync(fake, S0, r)
    for r in range(reps):
        S0.pfx = f"R{r}_"
        body_tensor(fake, S0, r)
    for r in range(reps):
        S0.pfx = f"R{r}_"
        body_scalar(fake, S0, r)
    for r in range(reps):
        S0.pfx = f"R{r}_"
        body_vector(fake, S0, r)

    S1 = Sched(sems, dry=False, ev=S0.ev)
    block = es.enter_context(nc.Block())

    @block.sync
    def _(sync):
        for r in range(reps):
            S1.pfx = f"R{r}_"
            body_sync(sync, S1, r)

    @block.tensor
    def _(tensor):
        for r in range(reps):
            S1.pfx = f"R{r}_"
            body_tensor(tensor, S1, r)

    @block.scalar
    def _(scalar):
        for r in range(reps):
            S1.pfx = f"R{r}_"
            body_scalar(scalar, S1, r)

    @block.vector
    def _(vector):
        for r in range(reps):
            S1.pfx = f"R{r}_"
            body_vector(vector, S1, r)

    es.close()
    return nc




# ---------------------------------------------------------------------------
# PJRT runner (built once, cached)
# ---------------------------------------------------------------------------
import jax
from jax.experimental.shard_map import shard_map
from jax.sharding import Mesh, PartitionSpec
from concourse.bass2jax import _bass_exec_p, partition_id_tensor, install_neuronx_cc_hook


def make_runner(nc, n_cores):

    install_neuronx_cc_hook()
    partition_name = nc.partition_id_tensor.name if nc.partition_id_tensor else None

    in_names, out_names, out_avals, zero_outs = [], [], [], []
    for alloc in nc.m.functions[0].allocations:
        if not isinstance(alloc, mybir.MemoryLocationSet):
            continue
        name = alloc.memorylocations[0].name
        if alloc.kind == "ExternalInput":
            if name != partition_name:
                in_names.append(name)
        elif alloc.kind == "ExternalOutput":
            out_names.append(name)
            shape = tuple(alloc.tensor_shape)
            dtype = mybir.dt.np(alloc.dtype)
            out_avals.append(jax.core.ShapedArray(shape, dtype))
            zero_outs.append(np.zeros(shape, dtype))
    n_params = len(in_names)
    n_outs = len(out_avals)
    all_in_names = list(in_names) + list(out_names)
    if partition_name is not None:
        all_in_names.append(partition_name)
    donate = tuple(range(n_params, n_params + n_outs))

    def _body(*args):
        operands = list(args)
        if partition_name is not None:
            operands.append(partition_id_tensor())
        outs = _bass_exec_p.bind(
            *operands,
            out_avals=tuple(out_avals),
            in_names=tuple(all_in_names),
            out_names=tuple(out_names),
            lowering_input_output_aliases=(),
            sim_require_finite=True,
            sim_require_nnan=True,
            nc=nc,
        )
        return tuple(outs)

    devices = jax.devices()[:n_cores]
    mesh = Mesh(np.asarray(devices), ("core",))
    in_specs = (PartitionSpec("core"),) * (n_params + n_outs)
    out_specs = (PartitionSpec("core"),) * n_outs
    fn = jax.jit(
        shard_map(_body, mesh=mesh, in_specs=in_specs, out_specs=out_specs,
                  check_rep=False),
        donate_argnums=donate, keep_unused=True,
    )

    def run(in_maps):
        global_ins = [
            np.concatenate([np.asarray(m[name]) for m in in_maps], axis=0)
            for name in in_names
        ]
        global_zeros = [
            np.concatenate([z] * n_cores, axis=0) for z in zero_outs
        ]
        out_arrs = fn(*global_ins, *global_zeros)
        out_splits = [np.split(np.asarray(a), n_cores, axis=0) for a in out_arrs]
        return [
            {name: out_splits[i][c] for i, name in enumerate(out_names)}
            for c in range(n_cores)
        ]

    run.out_names = out_names
    return run


_CACHE = {}


def _get_runner():
    if "run" not in _CACHE:
        nc = build_chamfer_nc()
        _CACHE["run"] = make_runner(nc, 8)
    return _CACHE["run"]


def _make_in_maps(output, real):
    import ml_dtypes
    NC = 8
    output = np.asarray(output, dtype=np.float32)
    real = np.asarray(real, dtype=np.float32)
    idv = np.eye(128, dtype=ml_dtypes.bfloat16)
    onesb = np.ones((1, N), ml_dtypes.bfloat16)
    in_maps = []
    for c in range(NC):
        sl = slice(c * SB, (c + 1) * SB)
        # [SB, N, D] -> [SB, D, N]
        p = np.ascontiguousarray(output[sl].transpose(0, 2, 1))
        r = np.ascontiguousarray(real[sl].transpose(0, 2, 1))
        in_maps.append({"pred": p, "real": r, "idb": idv, "cstb": onesb})
    return in_maps


def kernel(output, mu, log_var, real):
    B = 16
    NC = 8
    in_maps = _make_in_maps(output, real)
    outs = _get_runner()(in_maps)
    total = 0.0
    for c in range(NC):
        for b in range(SB):
            total += float(outs[c][f"res{b}"].sum())
    ch = -total / (16.0 * B * N)
    return np.array([ch], dtype=np.float32)

